# revision 3
# baseline (speedup 1.0000x reference)
"""Trainium2 Bass kernel for nn_BatchReLUTransformer (ReLU relaxation bound
propagation). Fully elementwise over (neuron, batch); batch dim (axis 1)
sharded across 8 NeuronCores, no communication.

Math (per element, l = bounds[...,0], u = bounds[...,1], l <= u, beta == 0):
  rnl   = relu(-l)                 (ScalarE)
  ru    = relu(u)                  (ScalarE)
  diff  = ru + rnl                 (== u-l on the unstable region)
  r     = 1/(diff + 3e-5)          (ScalarE reciprocal LUT, eps bias keeps
                                    r <= 3.4e4 so no f16 inf/NaN possible)
  lm    = ru * r                   (in [0,1]: == lmbda)
  out_u = min(ru, lm * (lu + rnl)) (== min(cur_u, lmbda*lu + mu))
  out_l = (l > 0) * max(l, ll)
General-beta inputs (and |x| > 3e4) fall back to an f32 path.

f16 fast path layout: the host packs, per tile t of F columns, the four
input planes [l|u|ll|lu] contiguously in one DRAM row-block so each tile is
ONE [128, 4F] DMA; outputs are packed [out_l|out_u] as one [128, 2F] store.
Engine placement: relu/recip on ScalarE, max(l,ll) on GpSimd (Pool),
everything else on VectorE at the 2x_1P f16 rate (tensor_scalar mask at 4x).
All f16 intermediates are bounded (inputs <= 3e4, r <= 3.4e4, lm <= 1), so
no inf/NaN can be produced for any finite input within the host gate; the
only divergence from the f32 reference is f16 quantization (l2 ~3.5e-4,
family gate 2e-2), verified over 30 input seeds plus an adversarial grid
of zeros/subnormals/huge values.
"""

import sys

import numpy as np

if "/opt/trn_rl_repo" not in sys.path:
    sys.path.insert(0, "/opt/trn_rl_repo")

N, B, M = 8192, 2048, 8
BS = B // M  # 256 batch entries per core
P = 128  # SBUF partitions
TOT = N * BS // P  # 16384 f16 elems per partition per plane

_CACHE = {}

# f16 fast-path schedule: columns per tile (sums to TOT). Small edge tiles
# shorten the pipeline ramp and the final store drain.
SCHED = [1024, 2048, 4096, 4096, 4096, 1024]
RECIP_EPS = 3e-5  # 1/eps = 3.3e4 < f16 max 65504


def _act_recip(nc, mybir, eng, out, in_, bias=0.0):
    ins = [eng.lower_ap(in_)]
    for arg in (bias, 1.0, 0.0):  # bias, scale, alpha
        ins.append(mybir.ImmediateValue(dtype=mybir.dt.float32, value=arg))
    eng.add_instruction(
        mybir.InstActivation(
            name=nc.get_next_instruction_name(),
            func=mybir.ActivationFunctionType.Reciprocal,
            ins=ins,
            outs=[eng.lower_ap(out)],
        )
    )


def _build_f16(io_bufs: int = 2, out_bufs: int = 2, tmp_bufs: int = 5,
               pool_mx: bool = False):
    """f16 fast path: packed per-tile loads/stores, beta==0 only."""
    import concourse.bacc as bacc
    import concourse.mybir as mybir
    import concourse.tile as tile

    Alu = mybir.AluOpType
    f32 = mybir.dt.float32
    f16 = mybir.dt.float16

    nc = bacc.Bacc(
        "TRN2", target_bir_lowering=False, debug=False, enable_asserts=False
    )
    in_d = nc.dram_tensor("inp", [P, 4 * TOT], f16, kind="ExternalInput").ap()
    out_d = nc.dram_tensor("out", [P, 2 * TOT], f16, kind="ExternalOutput").ap()

    with tile.TileContext(nc) as tc:
        with (
            tc.tile_pool(name="io", bufs=io_bufs) as io,
            tc.tile_pool(name="ot", bufs=out_bufs) as ot,
            tc.tile_pool(name="keep", bufs=2) as kp,
            tc.tile_pool(name="tmp", bufs=tmp_bufs) as tp,
        ):
            T = len(SCHED)
            offs = []
            o = 0
            for F in SCHED:
                offs.append((o, F))
                o += F
            assert o == TOT

            INs = {}
            RELUs = {}
            MXs = {}

            def load(t):
                o, F = offs[t]
                X = io.tile([P, 4 * F], f16, tag="IN", name=f"IN{t}")
                nc.sync.dma_start(out=X[:], in_=in_d[:, 4 * o : 4 * o + 4 * F])
                INs[t] = X

            def head(t):
                # ScalarE relus + Pool max for tile t; emitted one tile ahead
                # so both engines run ahead of the DVE chain.
                o, F = offs[t]
                X = INs[t]
                l = X[:, 0:F]
                u = X[:, F : 2 * F]
                ll = X[:, 2 * F : 3 * F]
                rnl = kp.tile([P, F], f16, tag="rnl", name=f"rnl{t}")[:]
                nc.scalar.activation(
                    rnl, l, mybir.ActivationFunctionType.Relu, scale=-1.0
                )
                ru = kp.tile([P, F], f16, tag="ru", name=f"ru{t}")[:]
                nc.scalar.activation(ru, u, mybir.ActivationFunctionType.Relu)
                RELUs[t] = (rnl, ru)
                if pool_mx:
                    mx = tp.tile([P, F], f16, tag="tmp", name=f"mx{t}")[:]
                    nc.gpsimd.tensor_tensor(mx, l, ll, op=Alu.max)
                    MXs[t] = mx

            # Prologue: first two loads go to the SP queue before anything
            # else; engine warm-ups (ACT table load, Pool ucode dispatch)
            # overlap the initial DMA ramp.
            load(0)
            load(1)
            if pool_mx:
                w1 = tp.tile([P, 2], f16, tag="warmp", name="warmp1")
                nc.gpsimd.memset(w1[:], 1.0)
                w2 = tp.tile([P, 2], f16, tag="warmp", name="warmp2")
                nc.gpsimd.tensor_tensor(w2[:], w1[:], w1[:], op=Alu.mult)
            warm = kp.tile([P, 1], f32, tag="warm", name="warm")
            _act_recip(nc, mybir, nc.scalar, warm[:], nc.const_aps.aps[(f32, 1.0)][:P])
            head(0)

            for t in range(T):
                o, F = offs[t]
                if t + 2 < T:
                    load(t + 2)
                X = INs.pop(t)
                l = X[:, 0:F]
                ll = X[:, 2 * F : 3 * F]
                lu = X[:, 3 * F : 4 * F]
                rnl, ru = RELUs.pop(t)

                def tmp(nm):
                    return tp.tile([P, F], f16, tag="tmp", name=f"{nm}{t}")[:]

                O = ot.tile([P, 2 * F], f16, tag="OUT", name=f"O{t}")
                # diff = ru + rnl ; r = 1/(diff + eps)  (ScalarE LUT)
                diff = tmp("diff")
                nc.vector.tensor_add(diff, ru, rnl)
                r = tmp("r")
                _act_recip(nc, mybir, nc.scalar, r, diff, bias=RECIP_EPS)
                # recip-independent DVE work hides the ACT recip latency
                tsum = tmp("tsum")
                nc.vector.tensor_add(tsum, lu, rnl)
                if not pool_mx:
                    mx = tmp("mx")
                    nc.vector.tensor_tensor(mx, l, ll, op=Alu.max)
                else:
                    mx = MXs.pop(t)
                m2 = tmp("m2")
                nc.vector.tensor_scalar(m2, l, 0.0, None, op0=Alu.is_gt)
                # out_l = (l>0) * max(l, ll)
                nc.vector.tensor_mul(O[:, 0:F], m2, mx)
                # next tile's ScalarE/Pool work queued before this tile's
                # store trigger so neither engine idles behind the DVE
                if t + 1 < T:
                    head(t + 1)
                # lmbda = ru*r in [0,1]; out_u = min(ru, lmbda*(lu+rnl))
                lm = tmp("lm")
                nc.vector.tensor_mul(lm, ru, r)
                v = tmp("v")
                nc.vector.tensor_mul(v, lm, tsum)
                nc.vector.tensor_tensor(O[:, F : 2 * F], ru, v, op=Alu.min)
                nc.scalar.dma_start(
                    out=out_d[:, 2 * o : 2 * o + 2 * F], in_=O[:]
                )

    nc.compile()
    return nc


def _build(with_beta: bool, F: int, tiles: int, io_bufs: int = 3, gpsimd_tt: bool = False):
    """f32 fallback path (nonzero beta, or inputs outside the f16 gate)."""
    import concourse.bacc as bacc
    import concourse.mybir as mybir
    import concourse.tile as tile

    Alu = mybir.AluOpType
    f32 = mybir.dt.float32

    nc = bacc.Bacc(
        "TRN2",
        target_bir_lowering=False,
        debug=False,
        enable_asserts=False,
    )
    # Register the tiny-eps bias const used by the rnl activation.
    EPS = 1e-30
    eps_t = nc.alloc_sbuf_tensor("const-f32-eps", [128, 1], f32)
    nc.gpsimd.memset(eps_t.ap(), EPS)
    nc.const_aps.aps[(f32, EPS)] = eps_t.ap()

    bounds_d = nc.dram_tensor(
        "bounds", [tiles, P, F, 2], f32, kind="ExternalInput"
    ).ap()
    last_d = nc.dram_tensor("last", [tiles, P, F, 2], f32, kind="ExternalInput").ap()
    beta_d = None
    if with_beta:
        beta_d = nc.dram_tensor("beta", [tiles, P, F], f32, kind="ExternalInput").ap()
    out_d = nc.dram_tensor("out", [tiles, P, F, 2], f32, kind="ExternalOutput").ap()

    with tile.TileContext(nc) as tc:
        with (
            tc.tile_pool(name="io", bufs=io_bufs) as io,
            tc.tile_pool(name="keep", bufs=2) as kp,
            tc.tile_pool(name="tmp", bufs=4) as tp,
        ):
            for t in range(tiles):
                X = io.tile([P, F, 2], f32, tag="X")
                nc.sync.dma_start(out=X[:], in_=bounds_d[t])
                Y = io.tile([P, F, 2], f32, tag="Y")
                nc.sync.dma_start(out=Y[:], in_=last_d[t])
                if with_beta:
                    BT = io.tile([P, F], f32, tag="BT")
                    nc.sync.dma_start(out=BT[:], in_=beta_d[t])

                l = X[:, :, 0]
                u = X[:, :, 1]
                ll = Y[:, :, 0]
                lu = Y[:, :, 1]

                cnt = iter(range(100))

                def tmp():
                    return tp.tile(
                        [P, F], f32, tag="tmp", name=f"tmp{t}_{next(cnt)}"
                    )[:]

                # ScalarE: rnl = relu(-l + 1e-30) (eps guards l==u==0 -> diff=0)
                rnl = kp.tile([P, F], f32, tag="rnl", name=f"rnl{t}")[:]
                nc.scalar.activation(
                    rnl, l, mybir.ActivationFunctionType.Relu, bias=1e-30, scale=-1.0
                )
                # ScalarE: ru = relu(u)
                ru = kp.tile([P, F], f32, tag="ru", name=f"ru{t}")[:]
                nc.scalar.activation(ru, u, mybir.ActivationFunctionType.Relu)
                # diff = ru + rnl ; r = 1/diff on ScalarE LUT (~1.2e-5 rel err)
                diff = tmp()
                nc.vector.tensor_add(diff, ru, rnl)
                r = tmp()
                _act_recip(nc, mybir, nc.scalar, r, diff)
                # recip-independent DVE work first (hides ACT recip latency)
                eng = nc.gpsimd if gpsimd_tt else nc.vector
                tsum = tmp()
                eng.tensor_add(tsum, lu, rnl)
                O = io.tile([P, F, 2], f32, tag="O", bufs=2)
                if not with_beta:
                    # nl = (l>0) * ll ; out_l = max(relu(l), nl)
                    nl = tmp()
                    nc.vector.scalar_tensor_tensor(
                        nl, l, 0.0, ll, op0=Alu.is_gt, op1=Alu.mult
                    )
                    nc.vector.scalar_tensor_tensor(
                        O[:, :, 0], l, 0.0, nl, op0=Alu.max, op1=Alu.max
                    )
                # lmbda = ru * r
                lm = tmp()
                nc.vector.tensor_mul(lm, ru, r)
                # v = lmbda * tsum  (== lmbda*lu + mu)
                v = tmp()
                eng.tensor_mul(v, lm, tsum)
                # out_u = min(ru, v)
                nc.vector.tensor_tensor(O[:, :, 1], ru, v, op=Alu.min)
                if with_beta:
                    # be = (l>0) + beta * ((u>0) - (l>0))
                    m2 = tmp()
                    nc.vector.tensor_scalar(m2, l, 0.0, None, op0=Alu.is_gt)
                    mgap = tmp()
                    nc.vector.scalar_tensor_tensor(
                        mgap, u, 0.0, m2, op0=Alu.is_gt, op1=Alu.subtract
                    )
                    bg = tmp()
                    nc.vector.tensor_mul(bg, BT[:], mgap)
                    be = tmp()
                    nc.vector.tensor_add(be, m2, bg)
                    # new_l = relu(be)*ll + min(be,0)*lu
                    t2 = tmp()
                    nc.vector.scalar_tensor_tensor(
                        t2, be, 0.0, ll, op0=Alu.max, op1=Alu.mult
                    )
                    bn = tmp()
                    nc.vector.scalar_tensor_tensor(
                        bn, be, 0.0, lu, op0=Alu.min, op1=Alu.mult
                    )
                    t4 = tmp()
                    nc.vector.tensor_add(t4, t2, bn)
                    nc.vector.scalar_tensor_tensor(
                        O[:, :, 0], l, 0.0, t4, op0=Alu.max, op1=Alu.max
                    )
                nc.scalar.dma_start(out=out_d[t], in_=O[:])

    nc.compile()
    return nc


VARIANT = {}       # f32-path experiment knobs
F16_VARIANT = {}   # f16-path experiment knobs, e.g. {"pool_mx": False}
USE_F16 = True


def _f16_safe(bounds, last_bounds):
    """True iff the f16 fast path is numerically safe: no f16 overflow on
    conversion. (NaN inputs fail the comparison and fall back too.) All
    other f16 hazards are handled structurally on-device."""
    return bool(
        (np.abs(bounds).max() <= 3.0e4)
        and (np.abs(last_bounds).max() <= 3.0e4)
    )


def _get(with_beta: bool):
    key = (with_beta, tuple(sorted(VARIANT.items())))
    if key not in _CACHE:
        F = 1024 if with_beta else 2048
        pairs = N * BS
        tiles = pairs // (P * F)
        assert tiles * P * F == pairs
        _CACHE[key] = (_build(with_beta, F, tiles, **VARIANT), F, tiles)
    return _CACHE[key]


def _get_f16():
    key = ("f16", tuple(sorted(F16_VARIANT.items())))
    if key not in _CACHE:
        assert sum(SCHED) == TOT
        _CACHE[key] = _build_f16(**F16_VARIANT)
    return _CACHE[key]


def _pack_f16(bounds, last_bounds, c):
    """Pack core c's shard into the tiled [P, 4*TOT] layout."""
    sl = slice(c * BS, (c + 1) * BS)
    pl = {
        0: bounds[:, sl, 0], 1: bounds[:, sl, 1],
        2: last_bounds[:, sl, 0], 3: last_bounds[:, sl, 1],
    }
    pl = {k: v.astype(np.float16).reshape(P, TOT) for k, v in pl.items()}
    inp = np.empty((P, 4 * TOT), dtype=np.float16)
    o = 0
    for F in SCHED:
        for k in range(4):
            inp[:, 4 * o + k * F : 4 * o + (k + 1) * F] = pl[k][:, o : o + F]
        o += F
    return inp


def _run_f16(bounds, last_bounds, trace=False):
    from concourse.bass_utils import run_bass_kernel_spmd

    nc = _get_f16()
    in_maps = [{"inp": _pack_f16(bounds, last_bounds, c)} for c in range(M)]
    res = run_bass_kernel_spmd(nc, in_maps, core_ids=list(range(M)), trace=trace)

    full = np.empty((N, B, 2), dtype=np.float32)
    for c, r in enumerate(res.results):
        sl = slice(c * BS, (c + 1) * BS)
        out = r["out"]
        ol = np.empty((P, TOT), np.float16)
        ou = np.empty((P, TOT), np.float16)
        o = 0
        for F in SCHED:
            ol[:, o : o + F] = out[:, 2 * o : 2 * o + F]
            ou[:, o : o + F] = out[:, 2 * o + F : 2 * o + 2 * F]
            o += F
        full[:, sl, 0] = ol.astype(np.float32).reshape(N, BS)
        full[:, sl, 1] = ou.astype(np.float32).reshape(N, BS)
    return full, res


def _run(bounds, beta, last_bounds, trace=False, force_f32=False):
    from concourse.bass_utils import run_bass_kernel_spmd

    bounds = np.ascontiguousarray(bounds, dtype=np.float32)
    last_bounds = np.ascontiguousarray(last_bounds, dtype=np.float32)
    beta = np.ascontiguousarray(beta, dtype=np.float32)
    with_beta = bool(np.any(beta))
    if (
        USE_F16
        and not with_beta
        and not force_f32
        and _f16_safe(bounds, last_bounds)
    ):
        return _run_f16(bounds, last_bounds, trace=trace)
    nc, F, tiles = _get(with_beta)

    in_maps = []
    for c in range(M):
        sl = slice(c * BS, (c + 1) * BS)
        m = {
            "bounds": np.ascontiguousarray(bounds[:, sl, :]).reshape(tiles, P, F, 2),
            "last": np.ascontiguousarray(last_bounds[:, sl, :]).reshape(tiles, P, F, 2),
        }
        if with_beta:
            m["beta"] = np.ascontiguousarray(beta[:, sl]).reshape(tiles, P, F)
        in_maps.append(m)

    res = run_bass_kernel_spmd(nc, in_maps, core_ids=list(range(M)), trace=trace)
    outs = [r["out"].reshape(N, BS, 2) for r in res.results]
    full = np.concatenate(outs, axis=1)
    return full, res


def kernel(bounds, beta, last_bounds):
    full, _ = _run(bounds, beta, last_bounds, trace=False)
    return full


# revision 6
# speedup vs baseline: 1.0370x; 1.0370x over previous
"""Trainium2 Bass kernel for nn_BatchReLUTransformer (ReLU relaxation bound
propagation). Fully elementwise over (neuron, batch); batch dim (axis 1)
sharded across 8 NeuronCores, no communication.

Math (per element, l = bounds[...,0], u = bounds[...,1], l <= u, beta == 0):
  rnl   = relu(-l)                 (ScalarE)
  ru    = relu(u)                  (ScalarE)
  diff  = ru + rnl                 (== u-l on the unstable region)
  r     = 1/(diff + 3e-5)          (ScalarE reciprocal LUT, eps bias keeps
                                    r <= 3.4e4 so no f16 inf/NaN possible)
  lm    = ru * r                   (in [0,1]: == lmbda)
  out_u = min(ru, lm * (lu + rnl)) (== min(cur_u, lmbda*lu + mu))
  out_l = (l > 0) * max(l, ll)
General-beta inputs (and |x| > 3e4) fall back to an f32 path.

f16 fast path layout: the host packs, per tile t of F columns, the four
input planes [l|u|ll|lu] contiguously in one DRAM row-block so each tile is
ONE [128, 4F] DMA; outputs are packed [out_l|out_u] as one [128, 2F] store.
Engine placement: relu/recip on ScalarE, max(l,ll) on GpSimd (Pool),
everything else on VectorE at the 2x_1P f16 rate (tensor_scalar mask at 4x).
All f16 intermediates are bounded (inputs <= 3e4, r <= 3.4e4, lm <= 1), so
no inf/NaN can be produced for any finite input within the host gate; the
only divergence from the f32 reference is f16 quantization (l2 ~3.5e-4,
family gate 2e-2), verified over 30 input seeds plus an adversarial grid
of zeros/subnormals/huge values.
"""

import sys

import numpy as np

if "/opt/trn_rl_repo" not in sys.path:
    sys.path.insert(0, "/opt/trn_rl_repo")

N, B, M = 8192, 2048, 8
BS = B // M  # 256 batch entries per core
P = 128  # SBUF partitions
TOT = N * BS // P  # 16384 f16 elems per partition per plane

_CACHE = {}

# f16 fast-path schedule: columns per tile (sums to TOT). Small edge tiles
# shorten the pipeline ramp and the final store drain.
SCHED = [1024, 2048, 2048, 2048, 2048, 2048, 2048, 2048, 1024]
RECIP_EPS = 3e-5  # 1/eps = 3.3e4 < f16 max 65504


def _act_recip(nc, mybir, eng, out, in_, bias=0.0):
    ins = [eng.lower_ap(in_)]
    for arg in (bias, 1.0, 0.0):  # bias, scale, alpha
        ins.append(mybir.ImmediateValue(dtype=mybir.dt.float32, value=arg))
    eng.add_instruction(
        mybir.InstActivation(
            name=nc.get_next_instruction_name(),
            func=mybir.ActivationFunctionType.Reciprocal,
            ins=ins,
            outs=[eng.lower_ap(out)],
        )
    )


def _build_f16(io_bufs: int = 4, out_bufs: int = 3, tmp_bufs: int = 5,
               prefetch: int = 4):
    """f16 fast path: per-tile [l|u] and [ll|lu] paired loads, packed
    [out_l|out_u] stores, beta==0 only."""
    import concourse.bacc as bacc
    import concourse.mybir as mybir
    import concourse.tile as tile

    Alu = mybir.AluOpType
    f32 = mybir.dt.float32
    f16 = mybir.dt.float16

    nc = bacc.Bacc(
        "TRN2", target_bir_lowering=False, debug=False, enable_asserts=False
    )
    ab_d = nc.dram_tensor("ab", [P, 2 * TOT], f16, kind="ExternalInput").ap()
    cd_d = nc.dram_tensor("cd", [P, 2 * TOT], f16, kind="ExternalInput").ap()
    out_d = nc.dram_tensor("out", [P, 2 * TOT], f16, kind="ExternalOutput").ap()

    with tile.TileContext(nc) as tc:
        with (
            tc.tile_pool(name="io", bufs=io_bufs) as io,
            tc.tile_pool(name="ot", bufs=out_bufs) as ot,
            tc.tile_pool(name="keep", bufs=2) as kp,
            tc.tile_pool(name="tmp", bufs=tmp_bufs) as tp,
        ):
            T = len(SCHED)
            offs = []
            o = 0
            for F in SCHED:
                offs.append((o, F))
                o += F
            assert o == TOT

            As = {}
            Bs = {}
            RELUs = {}

            def load(t):
                o, F = offs[t]
                sl = slice(2 * o, 2 * o + 2 * F)
                A = io.tile([P, 2 * F], f16, tag="A", name=f"A{t}")
                nc.sync.dma_start(out=A[:], in_=ab_d[:, sl])
                As[t] = A
                Bt = io.tile([P, 2 * F], f16, tag="B", name=f"B{t}")
                nc.sync.dma_start(out=Bt[:], in_=cd_d[:, sl])
                Bs[t] = Bt

            def relus(t):
                # ScalarE relus for tile t; gated only on the [l|u] load.
                o, F = offs[t]
                A = As[t]
                l = A[:, 0:F]
                u = A[:, F : 2 * F]
                rnl = kp.tile([P, F], f16, tag="rnl", name=f"rnl{t}")[:]
                nc.scalar.activation(
                    rnl, l, mybir.ActivationFunctionType.Relu, scale=-1.0
                )
                ru = kp.tile([P, F], f16, tag="ru", name=f"ru{t}")[:]
                nc.scalar.activation(ru, u, mybir.ActivationFunctionType.Relu)
                RELUs[t] = (rnl, ru)

            # Prologue: the first loads go to the SP queue before anything
            # else; the ACT table-load warm-up overlaps the DMA ramp.
            for t in range(min(prefetch, T)):
                load(t)
            warm = kp.tile([P, 1], f32, tag="warm", name="warm")
            _act_recip(nc, mybir, nc.scalar, warm[:], nc.const_aps.aps[(f32, 1.0)][:P])
            relus(0)

            for t in range(T):
                o, F = offs[t]
                if t + prefetch < T:
                    load(t + prefetch)
                A = As.pop(t)
                Bt = Bs.pop(t)
                l = A[:, 0:F]
                ll = Bt[:, 0:F]
                lu = Bt[:, F : 2 * F]
                rnl, ru = RELUs.pop(t)

                def tmp(nm):
                    return tp.tile([P, F], f16, tag="tmp", name=f"{nm}{t}")[:]

                O = ot.tile([P, 2 * F], f16, tag="OUT", name=f"O{t}")
                # Input-readers first so A/B buffers free early for prefetch.
                m2 = tmp("m2")
                nc.vector.tensor_scalar(m2, l, 0.0, None, op0=Alu.is_gt)
                mx = tmp("mx")
                nc.vector.tensor_tensor(mx, l, ll, op=Alu.max)
                # diff = ru + rnl ; r = 1/(diff + eps)  (ScalarE LUT)
                diff = tmp("diff")
                nc.vector.tensor_add(diff, ru, rnl)
                r = tmp("r")
                _act_recip(nc, mybir, nc.scalar, r, diff, bias=RECIP_EPS)
                # recip-independent DVE work hides the ACT recip latency
                tsum = tmp("tsum")
                nc.vector.tensor_add(tsum, lu, rnl)
                # out_l = (l>0) * max(l, ll)
                nc.vector.tensor_mul(O[:, 0:F], m2, mx)
                # next tile's ScalarE relus queued before this tile's store
                # trigger so ACT never idles behind the DVE
                if t + 1 < T:
                    relus(t + 1)
                # lmbda = ru*r in [0,1]; out_u = min(ru, lmbda*(lu+rnl))
                lm = tmp("lm")
                nc.vector.tensor_mul(lm, ru, r)
                v = tmp("v")
                nc.vector.tensor_mul(v, lm, tsum)
                nc.vector.tensor_tensor(O[:, F : 2 * F], ru, v, op=Alu.min)
                nc.scalar.dma_start(
                    out=out_d[:, 2 * o : 2 * o + 2 * F], in_=O[:]
                )

    nc.compile()
    return nc


def _build(with_beta: bool, F: int, tiles: int, io_bufs: int = 3, gpsimd_tt: bool = False):
    """f32 fallback path (nonzero beta, or inputs outside the f16 gate)."""
    import concourse.bacc as bacc
    import concourse.mybir as mybir
    import concourse.tile as tile

    Alu = mybir.AluOpType
    f32 = mybir.dt.float32

    nc = bacc.Bacc(
        "TRN2",
        target_bir_lowering=False,
        debug=False,
        enable_asserts=False,
    )
    # Register the tiny-eps bias const used by the rnl activation.
    EPS = 1e-30
    eps_t = nc.alloc_sbuf_tensor("const-f32-eps", [128, 1], f32)
    nc.gpsimd.memset(eps_t.ap(), EPS)
    nc.const_aps.aps[(f32, EPS)] = eps_t.ap()

    bounds_d = nc.dram_tensor(
        "bounds", [tiles, P, F, 2], f32, kind="ExternalInput"
    ).ap()
    last_d = nc.dram_tensor("last", [tiles, P, F, 2], f32, kind="ExternalInput").ap()
    beta_d = None
    if with_beta:
        beta_d = nc.dram_tensor("beta", [tiles, P, F], f32, kind="ExternalInput").ap()
    out_d = nc.dram_tensor("out", [tiles, P, F, 2], f32, kind="ExternalOutput").ap()

    with tile.TileContext(nc) as tc:
        with (
            tc.tile_pool(name="io", bufs=io_bufs) as io,
            tc.tile_pool(name="keep", bufs=2) as kp,
            tc.tile_pool(name="tmp", bufs=4) as tp,
        ):
            for t in range(tiles):
                X = io.tile([P, F, 2], f32, tag="X")
                nc.sync.dma_start(out=X[:], in_=bounds_d[t])
                Y = io.tile([P, F, 2], f32, tag="Y")
                nc.sync.dma_start(out=Y[:], in_=last_d[t])
                if with_beta:
                    BT = io.tile([P, F], f32, tag="BT")
                    nc.sync.dma_start(out=BT[:], in_=beta_d[t])

                l = X[:, :, 0]
                u = X[:, :, 1]
                ll = Y[:, :, 0]
                lu = Y[:, :, 1]

                cnt = iter(range(100))

                def tmp():
                    return tp.tile(
                        [P, F], f32, tag="tmp", name=f"tmp{t}_{next(cnt)}"
                    )[:]

                # ScalarE: rnl = relu(-l + 1e-30) (eps guards l==u==0 -> diff=0)
                rnl = kp.tile([P, F], f32, tag="rnl", name=f"rnl{t}")[:]
                nc.scalar.activation(
                    rnl, l, mybir.ActivationFunctionType.Relu, bias=1e-30, scale=-1.0
                )
                # ScalarE: ru = relu(u)
                ru = kp.tile([P, F], f32, tag="ru", name=f"ru{t}")[:]
                nc.scalar.activation(ru, u, mybir.ActivationFunctionType.Relu)
                # diff = ru + rnl ; r = 1/diff on ScalarE LUT (~1.2e-5 rel err)
                diff = tmp()
                nc.vector.tensor_add(diff, ru, rnl)
                r = tmp()
                _act_recip(nc, mybir, nc.scalar, r, diff)
                # recip-independent DVE work first (hides ACT recip latency)
                eng = nc.gpsimd if gpsimd_tt else nc.vector
                tsum = tmp()
                eng.tensor_add(tsum, lu, rnl)
                O = io.tile([P, F, 2], f32, tag="O", bufs=2)
                if not with_beta:
                    # nl = (l>0) * ll ; out_l = max(relu(l), nl)
                    nl = tmp()
                    nc.vector.scalar_tensor_tensor(
                        nl, l, 0.0, ll, op0=Alu.is_gt, op1=Alu.mult
                    )
                    nc.vector.scalar_tensor_tensor(
                        O[:, :, 0], l, 0.0, nl, op0=Alu.max, op1=Alu.max
                    )
                # lmbda = ru * r
                lm = tmp()
                nc.vector.tensor_mul(lm, ru, r)
                # v = lmbda * tsum  (== lmbda*lu + mu)
                v = tmp()
                eng.tensor_mul(v, lm, tsum)
                # out_u = min(ru, v)
                nc.vector.tensor_tensor(O[:, :, 1], ru, v, op=Alu.min)
                if with_beta:
                    # be = (l>0) + beta * ((u>0) - (l>0))
                    m2 = tmp()
                    nc.vector.tensor_scalar(m2, l, 0.0, None, op0=Alu.is_gt)
                    mgap = tmp()
                    nc.vector.scalar_tensor_tensor(
                        mgap, u, 0.0, m2, op0=Alu.is_gt, op1=Alu.subtract
                    )
                    bg = tmp()
                    nc.vector.tensor_mul(bg, BT[:], mgap)
                    be = tmp()
                    nc.vector.tensor_add(be, m2, bg)
                    # new_l = relu(be)*ll + min(be,0)*lu
                    t2 = tmp()
                    nc.vector.scalar_tensor_tensor(
                        t2, be, 0.0, ll, op0=Alu.max, op1=Alu.mult
                    )
                    bn = tmp()
                    nc.vector.scalar_tensor_tensor(
                        bn, be, 0.0, lu, op0=Alu.min, op1=Alu.mult
                    )
                    t4 = tmp()
                    nc.vector.tensor_add(t4, t2, bn)
                    nc.vector.scalar_tensor_tensor(
                        O[:, :, 0], l, 0.0, t4, op0=Alu.max, op1=Alu.max
                    )
                nc.scalar.dma_start(out=out_d[t], in_=O[:])

    nc.compile()
    return nc


VARIANT = {}       # f32-path experiment knobs
F16_VARIANT = {}   # f16-path experiment knobs, e.g. {"pool_mx": False}
USE_F16 = True


def _f16_safe(bounds, last_bounds):
    """True iff the f16 fast path is numerically safe: no f16 overflow on
    conversion. (NaN inputs fail the comparison and fall back too.) All
    other f16 hazards are handled structurally on-device."""
    return bool(
        (np.abs(bounds).max() <= 3.0e4)
        and (np.abs(last_bounds).max() <= 3.0e4)
    )


def _get(with_beta: bool):
    key = (with_beta, tuple(sorted(VARIANT.items())))
    if key not in _CACHE:
        F = 1024 if with_beta else 2048
        pairs = N * BS
        tiles = pairs // (P * F)
        assert tiles * P * F == pairs
        _CACHE[key] = (_build(with_beta, F, tiles, **VARIANT), F, tiles)
    return _CACHE[key]


def _get_f16():
    key = ("f16", tuple(sorted(F16_VARIANT.items())))
    if key not in _CACHE:
        assert sum(SCHED) == TOT
        _CACHE[key] = _build_f16(**F16_VARIANT)
    return _CACHE[key]


def _pack_pair(x, c):
    """Pack core c's shard of one (N, B, 2) tensor into per-tile [p0|p1]
    blocks laid out as [P, 2*TOT]."""
    sl = slice(c * BS, (c + 1) * BS)
    p0 = x[:, sl, 0].astype(np.float16).reshape(P, TOT)
    p1 = x[:, sl, 1].astype(np.float16).reshape(P, TOT)
    out = np.empty((P, 2 * TOT), dtype=np.float16)
    o = 0
    for F in SCHED:
        out[:, 2 * o : 2 * o + F] = p0[:, o : o + F]
        out[:, 2 * o + F : 2 * o + 2 * F] = p1[:, o : o + F]
        o += F
    return out


def _run_f16(bounds, last_bounds, trace=False):
    from concourse.bass_utils import run_bass_kernel_spmd

    nc = _get_f16()
    in_maps = [
        {"ab": _pack_pair(bounds, c), "cd": _pack_pair(last_bounds, c)}
        for c in range(M)
    ]
    res = run_bass_kernel_spmd(nc, in_maps, core_ids=list(range(M)), trace=trace)

    full = np.empty((N, B, 2), dtype=np.float32)
    for c, r in enumerate(res.results):
        sl = slice(c * BS, (c + 1) * BS)
        out = r["out"]
        ol = np.empty((P, TOT), np.float16)
        ou = np.empty((P, TOT), np.float16)
        o = 0
        for F in SCHED:
            ol[:, o : o + F] = out[:, 2 * o : 2 * o + F]
            ou[:, o : o + F] = out[:, 2 * o + F : 2 * o + 2 * F]
            o += F
        full[:, sl, 0] = ol.astype(np.float32).reshape(N, BS)
        full[:, sl, 1] = ou.astype(np.float32).reshape(N, BS)
    return full, res


def _run(bounds, beta, last_bounds, trace=False, force_f32=False):
    from concourse.bass_utils import run_bass_kernel_spmd

    bounds = np.ascontiguousarray(bounds, dtype=np.float32)
    last_bounds = np.ascontiguousarray(last_bounds, dtype=np.float32)
    beta = np.ascontiguousarray(beta, dtype=np.float32)
    with_beta = bool(np.any(beta))
    if (
        USE_F16
        and not with_beta
        and not force_f32
        and _f16_safe(bounds, last_bounds)
    ):
        return _run_f16(bounds, last_bounds, trace=trace)
    nc, F, tiles = _get(with_beta)

    in_maps = []
    for c in range(M):
        sl = slice(c * BS, (c + 1) * BS)
        m = {
            "bounds": np.ascontiguousarray(bounds[:, sl, :]).reshape(tiles, P, F, 2),
            "last": np.ascontiguousarray(last_bounds[:, sl, :]).reshape(tiles, P, F, 2),
        }
        if with_beta:
            m["beta"] = np.ascontiguousarray(beta[:, sl]).reshape(tiles, P, F)
        in_maps.append(m)

    res = run_bass_kernel_spmd(nc, in_maps, core_ids=list(range(M)), trace=trace)
    outs = [r["out"].reshape(N, BS, 2) for r in res.results]
    full = np.concatenate(outs, axis=1)
    return full, res


def kernel(bounds, beta, last_bounds):
    full, _ = _run(bounds, beta, last_bounds, trace=False)
    return full


# revision 8
# speedup vs baseline: 1.1205x; 1.0806x over previous
"""Trainium2 Bass kernel for nn_BatchReLUTransformer (ReLU relaxation bound
propagation). Fully elementwise over (neuron, batch); batch dim (axis 1)
sharded across 8 NeuronCores, no communication.

Math (per element, l = bounds[...,0], u = bounds[...,1], l <= u, beta == 0):
  rnl   = relu(-l)                 (ScalarE)
  ru    = relu(u)                  (ScalarE)
  diff  = ru + rnl                 (== u-l on the unstable region)
  r     = 1/(diff + 3e-5)          (ScalarE reciprocal LUT, eps bias keeps
                                    r <= 3.4e4 so no f16 inf/NaN possible)
  lm    = ru * r                   (in [0,1]: == lmbda)
  out_u = min(ru, lm * (lu + rnl)) (== min(cur_u, lmbda*lu + mu))
  out_l = (l > 0) * max(l, ll)
General-beta inputs (and |x| > 3e4) fall back to an f32 path.

f16 fast path layout: the host packs, per tile t of F columns, the four
input planes [l|u|ll|lu] contiguously in one DRAM row-block so each tile is
ONE [128, 4F] DMA; outputs are packed [out_l|out_u] as one [128, 2F] store.
Engine placement: relu/recip on ScalarE, max(l,ll) on GpSimd (Pool),
everything else on VectorE at the 2x_1P f16 rate (tensor_scalar mask at 4x).
All f16 intermediates are bounded (inputs <= 3e4, r <= 3.4e4, lm <= 1), so
no inf/NaN can be produced for any finite input within the host gate; the
only divergence from the f32 reference is f16 quantization (l2 ~3.5e-4,
family gate 2e-2), verified over 30 input seeds plus an adversarial grid
of zeros/subnormals/huge values.
"""

import sys

import numpy as np

if "/opt/trn_rl_repo" not in sys.path:
    sys.path.insert(0, "/opt/trn_rl_repo")

N, B, M = 8192, 2048, 8
BS = B // M  # 256 batch entries per core
P = 128  # SBUF partitions
TOT = N * BS // P  # 16384 f16 elems per partition per plane

_CACHE = {}

# f16 fast-path schedule: columns per tile (sums to TOT). Small edge tiles
# shorten the pipeline ramp and the final store drain.
SCHED = [512, 1024, 2048, 2048, 2048, 2048, 2048, 2048, 2048, 512]
RECIP_EPS = 3e-5  # 1/eps = 3.3e4 < f16 max 65504


def _act_recip(nc, mybir, eng, out, in_, bias=0.0):
    ins = [eng.lower_ap(in_)]
    for arg in (bias, 1.0, 0.0):  # bias, scale, alpha
        ins.append(mybir.ImmediateValue(dtype=mybir.dt.float32, value=arg))
    eng.add_instruction(
        mybir.InstActivation(
            name=nc.get_next_instruction_name(),
            func=mybir.ActivationFunctionType.Reciprocal,
            ins=ins,
            outs=[eng.lower_ap(out)],
        )
    )


def _build_f16(io_bufs: int = 4, out_bufs: int = 3, tmp_bufs: int = 5,
               prefetch: int = 4):
    """f16 fast path: per-tile [l|u] and [ll|lu] paired loads, packed
    [out_l|out_u] stores, beta==0 only."""
    import concourse.bacc as bacc
    import concourse.mybir as mybir
    import concourse.tile as tile

    Alu = mybir.AluOpType
    f32 = mybir.dt.float32
    f16 = mybir.dt.float16

    nc = bacc.Bacc(
        "TRN2", target_bir_lowering=False, debug=False, enable_asserts=False
    )
    ab_d = nc.dram_tensor("ab", [P, 2 * TOT], f16, kind="ExternalInput").ap()
    cd_d = nc.dram_tensor("cd", [P, 2 * TOT], f16, kind="ExternalInput").ap()
    out_d = nc.dram_tensor("out", [P, 2 * TOT], f16, kind="ExternalOutput").ap()

    with tile.TileContext(nc) as tc:
        with (
            tc.tile_pool(name="io", bufs=io_bufs) as io,
            tc.tile_pool(name="ot", bufs=out_bufs) as ot,
            tc.tile_pool(name="keep", bufs=2) as kp,
            tc.tile_pool(name="tmp", bufs=tmp_bufs) as tp,
        ):
            T = len(SCHED)
            offs = []
            o = 0
            for F in SCHED:
                offs.append((o, F))
                o += F
            assert o == TOT

            As = {}
            Bs = {}
            RELUs = {}

            def load(t):
                o, F = offs[t]
                sl = slice(2 * o, 2 * o + 2 * F)
                A = io.tile([P, 2 * F], f16, tag="A", name=f"A{t}")
                nc.sync.dma_start(out=A[:], in_=ab_d[:, sl])
                As[t] = A
                Bt = io.tile([P, 2 * F], f16, tag="B", name=f"B{t}")
                nc.sync.dma_start(out=Bt[:], in_=cd_d[:, sl])
                Bs[t] = Bt

            def relus(t):
                # ScalarE relus for tile t; gated only on the [l|u] load.
                o, F = offs[t]
                A = As[t]
                l = A[:, 0:F]
                u = A[:, F : 2 * F]
                rnl = kp.tile([P, F], f16, tag="rnl", name=f"rnl{t}")[:]
                nc.scalar.activation(
                    rnl, l, mybir.ActivationFunctionType.Relu, scale=-1.0
                )
                ru = kp.tile([P, F], f16, tag="ru", name=f"ru{t}")[:]
                nc.scalar.activation(ru, u, mybir.ActivationFunctionType.Relu)
                RELUs[t] = (rnl, ru)

            # Prologue: the first loads go to the SP queue before anything
            # else; the ACT table-load warm-up overlaps the DMA ramp.
            for t in range(min(prefetch, T)):
                load(t)
            warm = kp.tile([P, 1], f32, tag="warm", name="warm")
            _act_recip(nc, mybir, nc.scalar, warm[:], nc.const_aps.aps[(f32, 1.0)][:P])
            relus(0)

            for t in range(T):
                o, F = offs[t]
                if t + prefetch < T:
                    load(t + prefetch)
                A = As.pop(t)
                Bt = Bs.pop(t)
                l = A[:, 0:F]
                ll = Bt[:, 0:F]
                lu = Bt[:, F : 2 * F]
                rnl, ru = RELUs.pop(t)

                def tmp(nm):
                    return tp.tile([P, F], f16, tag="tmp", name=f"{nm}{t}")[:]

                O = ot.tile([P, 2 * F], f16, tag="OUT", name=f"O{t}")
                # diff first: its recip then runs on ScalarE under four
                # recip-independent DVE ops (m2, mx, tsum, OL), so lm never
                # waits on the ACT round-trip.
                diff = tmp("diff")
                nc.vector.tensor_add(diff, ru, rnl)
                r = tmp("r")
                _act_recip(nc, mybir, nc.scalar, r, diff, bias=RECIP_EPS)
                m2 = tmp("m2")
                nc.vector.tensor_scalar(m2, l, 0.0, None, op0=Alu.is_gt)
                mx = tmp("mx")
                nc.vector.tensor_tensor(mx, l, ll, op=Alu.max)
                tsum = tmp("tsum")
                nc.vector.tensor_add(tsum, lu, rnl)
                # out_l = (l>0) * max(l, ll)
                nc.vector.tensor_mul(O[:, 0:F], m2, mx)
                # next tile's ScalarE relus queued before this tile's store
                # trigger so ACT never idles behind the DVE
                if t + 1 < T:
                    relus(t + 1)
                # lmbda = ru*r in [0,1]; out_u = min(ru, lmbda*(lu+rnl))
                lm = tmp("lm")
                nc.vector.tensor_mul(lm, ru, r)
                v = tmp("v")
                nc.vector.tensor_mul(v, lm, tsum)
                nc.vector.tensor_tensor(O[:, F : 2 * F], ru, v, op=Alu.min)
                nc.scalar.dma_start(
                    out=out_d[:, 2 * o : 2 * o + 2 * F], in_=O[:]
                )

    nc.compile()
    return nc


def _build(with_beta: bool, F: int, tiles: int, io_bufs: int = 3, gpsimd_tt: bool = False):
    """f32 fallback path (nonzero beta, or inputs outside the f16 gate)."""
    import concourse.bacc as bacc
    import concourse.mybir as mybir
    import concourse.tile as tile

    Alu = mybir.AluOpType
    f32 = mybir.dt.float32

    nc = bacc.Bacc(
        "TRN2",
        target_bir_lowering=False,
        debug=False,
        enable_asserts=False,
    )
    # Register the tiny-eps bias const used by the rnl activation.
    EPS = 1e-30
    eps_t = nc.alloc_sbuf_tensor("const-f32-eps", [128, 1], f32)
    nc.gpsimd.memset(eps_t.ap(), EPS)
    nc.const_aps.aps[(f32, EPS)] = eps_t.ap()

    bounds_d = nc.dram_tensor(
        "bounds", [tiles, P, F, 2], f32, kind="ExternalInput"
    ).ap()
    last_d = nc.dram_tensor("last", [tiles, P, F, 2], f32, kind="ExternalInput").ap()
    beta_d = None
    if with_beta:
        beta_d = nc.dram_tensor("beta", [tiles, P, F], f32, kind="ExternalInput").ap()
    out_d = nc.dram_tensor("out", [tiles, P, F, 2], f32, kind="ExternalOutput").ap()

    with tile.TileContext(nc) as tc:
        with (
            tc.tile_pool(name="io", bufs=io_bufs) as io,
            tc.tile_pool(name="keep", bufs=2) as kp,
            tc.tile_pool(name="tmp", bufs=4) as tp,
        ):
            for t in range(tiles):
                X = io.tile([P, F, 2], f32, tag="X")
                nc.sync.dma_start(out=X[:], in_=bounds_d[t])
                Y = io.tile([P, F, 2], f32, tag="Y")
                nc.sync.dma_start(out=Y[:], in_=last_d[t])
                if with_beta:
                    BT = io.tile([P, F], f32, tag="BT")
                    nc.sync.dma_start(out=BT[:], in_=beta_d[t])

                l = X[:, :, 0]
                u = X[:, :, 1]
                ll = Y[:, :, 0]
                lu = Y[:, :, 1]

                cnt = iter(range(100))

                def tmp():
                    return tp.tile(
                        [P, F], f32, tag="tmp", name=f"tmp{t}_{next(cnt)}"
                    )[:]

                # ScalarE: rnl = relu(-l + 1e-30) (eps guards l==u==0 -> diff=0)
                rnl = kp.tile([P, F], f32, tag="rnl", name=f"rnl{t}")[:]
                nc.scalar.activation(
                    rnl, l, mybir.ActivationFunctionType.Relu, bias=1e-30, scale=-1.0
                )
                # ScalarE: ru = relu(u)
                ru = kp.tile([P, F], f32, tag="ru", name=f"ru{t}")[:]
                nc.scalar.activation(ru, u, mybir.ActivationFunctionType.Relu)
                # diff = ru + rnl ; r = 1/diff on ScalarE LUT (~1.2e-5 rel err)
                diff = tmp()
                nc.vector.tensor_add(diff, ru, rnl)
                r = tmp()
                _act_recip(nc, mybir, nc.scalar, r, diff)
                # recip-independent DVE work first (hides ACT recip latency)
                eng = nc.gpsimd if gpsimd_tt else nc.vector
                tsum = tmp()
                eng.tensor_add(tsum, lu, rnl)
                O = io.tile([P, F, 2], f32, tag="O", bufs=2)
                if not with_beta:
                    # nl = (l>0) * ll ; out_l = max(relu(l), nl)
                    nl = tmp()
                    nc.vector.scalar_tensor_tensor(
                        nl, l, 0.0, ll, op0=Alu.is_gt, op1=Alu.mult
                    )
                    nc.vector.scalar_tensor_tensor(
                        O[:, :, 0], l, 0.0, nl, op0=Alu.max, op1=Alu.max
                    )
                # lmbda = ru * r
                lm = tmp()
                nc.vector.tensor_mul(lm, ru, r)
                # v = lmbda * tsum  (== lmbda*lu + mu)
                v = tmp()
                eng.tensor_mul(v, lm, tsum)
                # out_u = min(ru, v)
                nc.vector.tensor_tensor(O[:, :, 1], ru, v, op=Alu.min)
                if with_beta:
                    # be = (l>0) + beta * ((u>0) - (l>0))
                    m2 = tmp()
                    nc.vector.tensor_scalar(m2, l, 0.0, None, op0=Alu.is_gt)
                    mgap = tmp()
                    nc.vector.scalar_tensor_tensor(
                        mgap, u, 0.0, m2, op0=Alu.is_gt, op1=Alu.subtract
                    )
                    bg = tmp()
                    nc.vector.tensor_mul(bg, BT[:], mgap)
                    be = tmp()
                    nc.vector.tensor_add(be, m2, bg)
                    # new_l = relu(be)*ll + min(be,0)*lu
                    t2 = tmp()
                    nc.vector.scalar_tensor_tensor(
                        t2, be, 0.0, ll, op0=Alu.max, op1=Alu.mult
                    )
                    bn = tmp()
                    nc.vector.scalar_tensor_tensor(
                        bn, be, 0.0, lu, op0=Alu.min, op1=Alu.mult
                    )
                    t4 = tmp()
                    nc.vector.tensor_add(t4, t2, bn)
                    nc.vector.scalar_tensor_tensor(
                        O[:, :, 0], l, 0.0, t4, op0=Alu.max, op1=Alu.max
                    )
                nc.scalar.dma_start(out=out_d[t], in_=O[:])

    nc.compile()
    return nc


VARIANT = {}       # f32-path experiment knobs
F16_VARIANT = {}   # f16-path experiment knobs, e.g. {"pool_mx": False}
USE_F16 = True


def _f16_safe(bounds, last_bounds):
    """True iff the f16 fast path is numerically safe: no f16 overflow on
    conversion. (NaN inputs fail the comparison and fall back too.) All
    other f16 hazards are handled structurally on-device."""
    return bool(
        (np.abs(bounds).max() <= 3.0e4)
        and (np.abs(last_bounds).max() <= 3.0e4)
    )


def _get(with_beta: bool):
    key = (with_beta, tuple(sorted(VARIANT.items())))
    if key not in _CACHE:
        F = 1024 if with_beta else 2048
        pairs = N * BS
        tiles = pairs // (P * F)
        assert tiles * P * F == pairs
        _CACHE[key] = (_build(with_beta, F, tiles, **VARIANT), F, tiles)
    return _CACHE[key]


def _get_f16():
    key = ("f16", tuple(sorted(F16_VARIANT.items())))
    if key not in _CACHE:
        assert sum(SCHED) == TOT
        _CACHE[key] = _build_f16(**F16_VARIANT)
    return _CACHE[key]


def _pack_pair(x, c):
    """Pack core c's shard of one (N, B, 2) tensor into per-tile [p0|p1]
    blocks laid out as [P, 2*TOT]."""
    sl = slice(c * BS, (c + 1) * BS)
    p0 = x[:, sl, 0].astype(np.float16).reshape(P, TOT)
    p1 = x[:, sl, 1].astype(np.float16).reshape(P, TOT)
    out = np.empty((P, 2 * TOT), dtype=np.float16)
    o = 0
    for F in SCHED:
        out[:, 2 * o : 2 * o + F] = p0[:, o : o + F]
        out[:, 2 * o + F : 2 * o + 2 * F] = p1[:, o : o + F]
        o += F
    return out


def _run_f16(bounds, last_bounds, trace=False):
    from concourse.bass_utils import run_bass_kernel_spmd

    nc = _get_f16()
    in_maps = [
        {"ab": _pack_pair(bounds, c), "cd": _pack_pair(last_bounds, c)}
        for c in range(M)
    ]
    res = run_bass_kernel_spmd(nc, in_maps, core_ids=list(range(M)), trace=trace)

    full = np.empty((N, B, 2), dtype=np.float32)
    for c, r in enumerate(res.results):
        sl = slice(c * BS, (c + 1) * BS)
        out = r["out"]
        ol = np.empty((P, TOT), np.float16)
        ou = np.empty((P, TOT), np.float16)
        o = 0
        for F in SCHED:
            ol[:, o : o + F] = out[:, 2 * o : 2 * o + F]
            ou[:, o : o + F] = out[:, 2 * o + F : 2 * o + 2 * F]
            o += F
        full[:, sl, 0] = ol.astype(np.float32).reshape(N, BS)
        full[:, sl, 1] = ou.astype(np.float32).reshape(N, BS)
    return full, res


def _run(bounds, beta, last_bounds, trace=False, force_f32=False):
    from concourse.bass_utils import run_bass_kernel_spmd

    bounds = np.ascontiguousarray(bounds, dtype=np.float32)
    last_bounds = np.ascontiguousarray(last_bounds, dtype=np.float32)
    beta = np.ascontiguousarray(beta, dtype=np.float32)
    with_beta = bool(np.any(beta))
    if (
        USE_F16
        and not with_beta
        and not force_f32
        and _f16_safe(bounds, last_bounds)
    ):
        return _run_f16(bounds, last_bounds, trace=trace)
    nc, F, tiles = _get(with_beta)

    in_maps = []
    for c in range(M):
        sl = slice(c * BS, (c + 1) * BS)
        m = {
            "bounds": np.ascontiguousarray(bounds[:, sl, :]).reshape(tiles, P, F, 2),
            "last": np.ascontiguousarray(last_bounds[:, sl, :]).reshape(tiles, P, F, 2),
        }
        if with_beta:
            m["beta"] = np.ascontiguousarray(beta[:, sl]).reshape(tiles, P, F)
        in_maps.append(m)

    res = run_bass_kernel_spmd(nc, in_maps, core_ids=list(range(M)), trace=trace)
    outs = [r["out"].reshape(N, BS, 2) for r in res.results]
    full = np.concatenate(outs, axis=1)
    return full, res


def kernel(bounds, beta, last_bounds):
    full, _ = _run(bounds, beta, last_bounds, trace=False)
    return full
